# revision 7
# baseline (speedup 1.0000x reference)
"""Luong attention kernel for Trainium2 (Bass/Tile), data-parallel over batch.

Math (per batch b):
    scores[s,t] = enc[s,:] . dec[t,:]
    weights     = softmax(scores, axis=t)
    context[s]  = sum_t weights[s,t] * enc[t,:]
    out         = tanh(concat([context, dec]) @ W_tanh)

Implementation notes (v2):
  - B=8 batches -> 8 NeuronCores, one batch per core, no collectives.
  - scoresT[t,s] is computed (t on partitions) so the context contraction
    over t maps onto the PE (lhsT = enc natural, rhs = exp(scoresT)).
  - softmax uses a *global* shift (softmax is shift-invariant): E = exp(s-64).
    Scores ~ N(0, 256): row max is ~[45..95], so exp(s-64) stays inside
    bf16 range on both ends; E is kept unnormalized and 1/denom is applied
    in the output matmul epilogue, where denom is a per-partition scalar.
  - denom: E chunks are pairwise-summed with a DVE+GPSIMD tree (keeps the
    vector engine under the PE roofline), then folded across partitions
    with 16 tiny f32 PE matmuls against a ones vector -> [s-part, 1].
  - phase 3 fuses (y1 * rden + y2) into one scalar_tensor_tensor, then
    tanh on ACT; per-block output DMA overlaps the next block.
  - startup: enc/dec are loaded in halves, cast f32->f16 on DVE, bounced
    through DRAM scratch and DMA-transposed per half so the first scores
    matmuls start while the tail of the input is still in flight.
"""

import sys

if "/opt/trn_rl_repo" not in sys.path:
    sys.path.insert(0, "/opt/trn_rl_repo")

import numpy as np

import concourse.bacc as bacc
import concourse.mybir as mybir
import concourse.tile as tile
from concourse import bass_utils

B, S, D = 8, 2048, 256
P = 128
NT = S // P  # 16 chunks of 128 along t (and s for output rows)
SB = 512  # moving-dim block for the big matmuls
NSB = S // SB  # 4
DC = D // P  # 2 partition chunks of the feature dim
NH = NT // 2  # n-chunks per half
SHIFT = 64.0  # global softmax shift

_CACHE = {}


def _build(reps: int = 1):
    f32, bf16, f16 = mybir.dt.float32, mybir.dt.bfloat16, mybir.dt.float16

    nc = bacc.Bacc("TRN2", target_bir_lowering=False, debug=False)
    enc_d = nc.dram_tensor("enc", [S, D], f32, kind="ExternalInput").ap()
    dec_d = nc.dram_tensor("dec", [S, D], f32, kind="ExternalInput").ap()
    w_d = nc.dram_tensor("w", [2 * D, D], f32, kind="ExternalInput").ap()
    out_d = nc.dram_tensor("out", [S, D], f32, kind="ExternalOutput").ap()

    with tile.TileContext(nc) as tc:
        with tc.tile_pool(name="big", bufs=1) as big:
            encT = big.tile([P, DC, S], f16, tag="encT")  # enc^T  (d-part, s-free)
            decT = big.tile([P, DC, S], f16, tag="decT")  # dec^T  (d-part, t-free)
            encN = big.tile([P, NT, D], bf16, tag="encN")  # enc natural, per t-chunk
            E = big.tile([P, NT, S], bf16, tag="E")  # exp(scoresT - SHIFT)
            U = big.tile([P, DC, S], bf16, tag="U")  # unnormalized context^T
            Wt1 = big.tile([P, DC, D], bf16, tag="Wt1")  # W_tanh rows 0..255 (ctx)
            Wt2 = big.tile([P, DC, D], f16, tag="Wt2")  # W_tanh rows 256..511 (dec)
            ones = big.tile([P, 1], f32, tag="ones")
            rden = big.tile([P, NT], f32, tag="rden")  # 1/denom, [s-part, s-chunk]
            nshift = big.tile([P, 1], f32, tag="nshift")
            outS = big.tile([P, NT, D], f32, tag="outS")  # staged output rows

            nc.any.memset(ones[:], 1.0)
            nc.any.memset(nshift[:], -SHIFT)

            for _rep in range(reps):
                _body(nc, tc, big, locals(), warmup=(_rep == 0))

    nc.compile()
    return nc


def _body(nc, tc, big, env, warmup=True):
    f32, bf16, f16 = mybir.dt.float32, mybir.dt.bfloat16, mybir.dt.float16
    AF = mybir.ActivationFunctionType
    ALU = mybir.AluOpType
    enc_d, dec_d, w_d, out_d = env["enc_d"], env["dec_d"], env["w_d"], env["out_d"]
    encT, decT, encN, E = env["encT"], env["decT"], env["encN"], env["E"]
    U, Wt1, Wt2 = env["U"], env["Wt1"], env["Wt2"]
    ones, rden, nshift, outS = env["ones"], env["rden"], env["nshift"], env["outS"]

    out_r = out_d.rearrange("(n p) d -> p n d", p=P)

    # ---- W: one batched DMA; rows 0..255 -> bf16 (ctx), 256..511 -> f16
    with tc.tile_pool(name="wstage", bufs=1) as wstage:
        wst = wstage.tile([P, 4, D], f32, tag="wst")
        nc.sync.dma_start(wst[:], w_d.rearrange("(r p) d -> p r d", p=P))
        for r in range(2):
            nc.vector.tensor_copy(Wt1[:, r, :], wst[:, r, :])
            nc.vector.tensor_copy(Wt2[:, r, :], wst[:, 2 + r, :])

    # ---- transposed operands: load f32 halves, cast to f16, bounce via
    # DRAM scratch, DMA-transpose per (half, d-chunk). enc natural is also
    # cast to bf16 for the context matmul.
    with (
        tc.tile_pool(name="scr", bufs=1, space="DRAM") as scr,
        tc.tile_pool(name="stS", bufs=3) as stS,
        tc.tile_pool(name="stH", bufs=3) as stH,
    ):
        scrE = scr.tile([S, D], f16, tag="scrE")
        scrD = scr.tile([S, D], f16, tag="scrD")
        for h in range(2):
            n0, r0 = h * NH, h * NH * P
            for src_d, scrX, dstT, natural in (
                (enc_d, scrE, encT, True),
                (dec_d, scrD, decT, False),
            ):
                xS = stS.tile([P, NH, D], f32, tag="xS")
                nc.sync.dma_start(
                    xS[:], src_d.rearrange("(n p) d -> p n d", p=P)[:, n0 : n0 + NH, :]
                )
                xH = stH.tile([P, NH, D], f16, tag="xH")
                nc.vector.tensor_copy(xH[:], xS[:])
                if natural:
                    nc.vector.tensor_copy(encN[:, n0 : n0 + NH, :], xS[:])
                nc.sync.dma_start(
                    scrX.rearrange("(n p) d -> p n d", p=P)[:, n0 : n0 + NH, :], xH[:]
                )
                for dc in range(DC):
                    nc.sync.dma_start(
                        out=dstT[:, dc, r0 : r0 + NH * P],
                        in_=scrX[r0 : r0 + NH * P, dc * P : (dc + 1) * P],
                        transpose=True,
                    )

    # ---- PE warmup: ~4us of dummy matmuls on W tiles during the input
    # DMA so the HAM clock-gate opens before the first real matmul.
    if warmup:
        with tc.tile_pool(name="ps_w", bufs=1, space="PSUM") as ps_w:
            pw = ps_w.tile([P, D], f32, tag="pw")
            for _ in range(20):
                nc.tensor.matmul(
                    pw[:], Wt1[:, 0, 0:P], Wt1[:, 0, :], start=True, stop=True
                )

    # ---- phase 1: scores + exp, t-outer. For each t-chunk: one LDW per
    # d-chunk covers 2 matmuls (two s-blocks into one 2-bank PSUM tile);
    # exp then covers 1024 columns per instruction.
    with (
        tc.tile_pool(name="ps_d", bufs=1, space="PSUM") as ps_d,
        tc.tile_pool(name="trP", bufs=6) as trP,
        tc.tile_pool(name="trQ", bufs=4) as trQ,
        tc.tile_pool(name="trR", bufs=4) as trR,
    ):
        pd = ps_d.tile([P, NT], f32, tag="pd")
        with tc.tile_pool(name="ps_s", bufs=2, space="PSUM") as ps_s:
            for t in range(NT):
                for half in range(2):
                    ps2 = ps_s.tile([P, 2, SB], f32, tag="ps2")
                    for dc in range(DC):
                        for sbh in range(2):
                            nc.tensor.matmul(
                                ps2[:, sbh, :],
                                decT[:, dc, t * P : (t + 1) * P],
                                encT[:, dc, (2 * half + sbh) * SB : (2 * half + sbh + 1) * SB],
                                start=(dc == 0),
                                stop=(dc == DC - 1),
                            )
                    nc.scalar.activation(
                        E[:, t, half * 2 * SB : (half + 1) * 2 * SB],
                        ps2[:, :, :],
                        AF.Exp,
                        bias=nshift[:],
                    )

        # denom tree over t-chunks, per s-half (FD=1024). Imbalanced so the
        # final combine after the last exp is short: A=sum(t0..7) closes at
        # t=7, B=sum(t8..13) at t=13, C=E14+E15 at t=15.
        HB = 2 * SB  # 1024
        esums = []
        for h in range(2):
            sl = slice(h * HB, (h + 1) * HB)
            l1 = []
            for i in range(7):
                p_i = trP.tile([P, HB], bf16, tag="p", name=f"p{h}_{i}")
                nc.vector.tensor_add(p_i[:], E[:, 2 * i, sl], E[:, 2 * i + 1, sl])
                l1.append(p_i)
            qa = trQ.tile([P, HB], f32, tag="q", name=f"qa{h}")
            nc.gpsimd.tensor_add(qa[:], l1[0][:], l1[1][:])
            qb = trQ.tile([P, HB], f32, tag="q", name=f"qb{h}")
            nc.gpsimd.tensor_add(qb[:], l1[2][:], l1[3][:])
            qc = trQ.tile([P, HB], f32, tag="q", name=f"qc{h}")
            nc.gpsimd.tensor_add(qc[:], l1[4][:], l1[5][:])
            ra = trR.tile([P, HB], f32, tag="r", name=f"ra{h}")
            nc.vector.tensor_add(ra[:], qa[:], qb[:])  # sum t0..7, ready ~t=7
            rb = trR.tile([P, HB], f32, tag="r", name=f"rb{h}")
            nc.vector.tensor_add(rb[:], ra[:], qc[:])  # sum t0..11
            rc = trR.tile([P, HB], f32, tag="r", name=f"rc{h}")
            nc.vector.tensor_add(rc[:], rb[:], l1[6][:])  # sum t0..13
            # C = E14+E15 lands last; fold it in with one f32 add
            p_c = trP.tile([P, HB], bf16, tag="p", name=f"pc{h}")
            nc.vector.tensor_add(p_c[:], E[:, 14, sl], E[:, 15, sl])
            esum = trR.tile([P, HB], f32, tag="r", name=f"esum{h}")
            nc.vector.tensor_add(esum[:], rc[:], p_c[:])
            esums.append(esum)

        # fold across partitions (16 tiny f32 matmuls) + reciprocal
        for c in range(NT):
            h, cc = c // 8, c % 8
            nc.tensor.matmul(
                pd[:, c : c + 1],
                esums[h][:, cc * P : (cc + 1) * P],
                ones[:],
                start=True,
                stop=True,
            )
        nc.vector.reciprocal(rden[:], pd[:])

        # ---- phase 2: context U = encN^T @ E. One LDW per (pair, dc, t)
        # covers 2 matmuls (two s-blocks into one 2-bank PSUM tile).
        with tc.tile_pool(name="ps_u", bufs=2, space="PSUM") as ps_u:
            for pair in range(2):
                for dc in range(DC):
                    pu2 = ps_u.tile([P, 2, SB], f32, tag="pu2")
                    for t in range(NT):
                        for sbh in range(2):
                            nc.tensor.matmul(
                                pu2[:, sbh, :],
                                encN[:, t, dc * P : (dc + 1) * P],
                                E[:, t, (2 * pair + sbh) * SB : (2 * pair + sbh + 1) * SB],
                                start=(t == 0),
                                stop=(t == NT - 1),
                            )
                    nc.vector.tensor_copy(
                        U[:, dc, pair * 2 * SB : (pair + 1) * 2 * SB], pu2[:, :, :]
                    )

        # ---- phase 3: out = tanh(U^T@W1 * rden + dec@W2); tanh batched
        # over groups of 4 s-chunks, per-group output DMA.
        with (
            tc.tile_pool(name="ps_y", bufs=4, space="PSUM") as ps_y,
            tc.tile_pool(name="fout", bufs=2) as fout,
        ):
            for g in range(4):
                ft = fout.tile([P, 4, D], f32, tag="ft")
                for c4 in range(4):
                    c = g * 4 + c4
                    py = ps_y.tile([P, 2, D], f32, tag="py")
                    for dc in range(DC):
                        nc.tensor.matmul(
                            py[:, 0, :],
                            U[:, dc, c * P : (c + 1) * P],
                            Wt1[:, dc, :],
                            start=(dc == 0),
                            stop=(dc == DC - 1),
                        )
                    for dc in range(DC):
                        nc.tensor.matmul(
                            py[:, 1, :],
                            decT[:, dc, c * P : (c + 1) * P],
                            Wt2[:, dc, :],
                            start=(dc == 0),
                            stop=(dc == DC - 1),
                        )
                    t1 = fout.tile([P, D], f32, tag="t1", name="t1")
                    nc.vector.tensor_scalar_mul(t1[:], py[:, 0, :], rden[:, c : c + 1])
                    nc.vector.tensor_add(ft[:, c4, :], t1[:], py[:, 1, :])
                nc.scalar.activation(outS[:, g * 4 : (g + 1) * 4, :], ft[:, :, :], AF.Tanh)
                nc.sync.dma_start(
                    out_r[:, g * 4 : g * 4 + 4, :], outS[:, g * 4 : g * 4 + 4, :]
                )


def get_nc():
    if "nc" not in _CACHE:
        _CACHE["nc"] = _build()
    return _CACHE["nc"]


def _get_fn():
    """Build the sharded PJRT executable once and cache it; subsequent
    kernel() calls pay only input transfer + dispatch."""
    if "fn" in _CACHE:
        return _CACHE["fn"]
    import jax
    from jax.sharding import Mesh, NamedSharding, PartitionSpec
    from jax.experimental.shard_map import shard_map
    from concourse.bass2jax import (
        _bass_exec_p,
        install_neuronx_cc_hook,
        partition_id_tensor,
    )

    install_neuronx_cc_hook()
    nc = get_nc()
    out_avals = []
    for alloc in nc.m.functions[0].allocations:
        if (
            isinstance(alloc, mybir.MemoryLocationSet)
            and alloc.kind == "ExternalOutput"
        ):
            out_avals.append(
                jax.core.ShapedArray(
                    tuple(alloc.tensor_shape), mybir.dt.np(alloc.dtype)
                )
            )
    has_pid = nc.partition_id_tensor is not None
    names = ["enc", "dec", "w", "out"] + (["partition_id"] if has_pid else [])
    mesh = Mesh(np.asarray(jax.devices()[:B]), ("core",))
    spec = PartitionSpec("core")

    def _b(e, d, ww, z):
        ops = [e, d, ww, z] + ([partition_id_tensor()] if has_pid else [])
        return _bass_exec_p.bind(
            *ops,
            out_avals=tuple(out_avals),
            in_names=tuple(names),
            out_names=("out",),
            lowering_input_output_aliases=(),
            sim_require_finite=True,
            sim_require_nnan=True,
            nc=nc,
        )[0]

    jitted = jax.jit(
        shard_map(
            _b, mesh=mesh, in_specs=(spec,) * 4, out_specs=spec, check_rep=False
        ),
        donate_argnums=(3,),
        keep_unused=True,
    )
    sh = NamedSharding(mesh, spec)
    _CACHE["fn"] = (jitted, sh)
    return _CACHE["fn"]


def kernel(enc_outputs_top, dec_outputs_top, W_tanh):
    import jax

    enc = np.ascontiguousarray(enc_outputs_top, dtype=np.float32)
    dec = np.ascontiguousarray(dec_outputs_top, dtype=np.float32)
    w = np.ascontiguousarray(W_tanh, dtype=np.float32)
    try:
        fn, sh = _get_fn()
        eg = jax.device_put(enc.reshape(B * S, D), sh)
        dg = jax.device_put(dec.reshape(B * S, D), sh)
        wg = jax.device_put(np.concatenate([w] * B, axis=0), sh)
        zg = jax.device_put(np.zeros((B * S, D), np.float32), sh)
        out = np.asarray(jax.block_until_ready(fn(eg, dg, wg, zg)))
        return out.reshape(B, S, D)
    except Exception:
        # fallback: reference multi-core path (rebuilds the jit per call)
        nc = get_nc()
        in_maps = [{"enc": enc[b], "dec": dec[b], "w": w} for b in range(B)]
        res = bass_utils.run_bass_kernel_spmd(nc, in_maps, core_ids=list(range(B)))
        return np.stack([r["out"] for r in res.results], axis=0)


# revision 8
# speedup vs baseline: 2.6148x; 2.6148x over previous
"""Luong attention kernel for Trainium2 (Bass/Tile), data-parallel over batch.

Math (per batch b):
    scores[s,t] = enc[s,:] . dec[t,:]
    weights     = softmax(scores, axis=t)
    context[s]  = sum_t weights[s,t] * enc[t,:]
    out         = tanh(concat([context, dec]) @ W_tanh)

Implementation notes (v2):
  - B=8 batches -> 8 NeuronCores, one batch per core, no collectives.
  - scoresT[t,s] is computed (t on partitions) so the context contraction
    over t maps onto the PE (lhsT = enc natural, rhs = exp(scoresT)).
  - softmax uses a *global* shift (softmax is shift-invariant): E = exp(s-64).
    Scores ~ N(0, 256): row max is ~[45..95], so exp(s-64) stays inside
    bf16 range on both ends; E is kept unnormalized and 1/denom is applied
    in the output matmul epilogue, where denom is a per-partition scalar.
  - denom: E chunks are pairwise-summed with a DVE+GPSIMD tree (keeps the
    vector engine under the PE roofline), then folded across partitions
    with 16 tiny f32 PE matmuls against a ones vector -> [s-part, 1].
  - phase 3 fuses (y1 * rden + y2) into one scalar_tensor_tensor, then
    tanh on ACT; per-block output DMA overlaps the next block.
  - startup: enc/dec are loaded in halves, cast f32->f16 on DVE, bounced
    through DRAM scratch and DMA-transposed per half so the first scores
    matmuls start while the tail of the input is still in flight.
"""

import sys

if "/opt/trn_rl_repo" not in sys.path:
    sys.path.insert(0, "/opt/trn_rl_repo")

import numpy as np

import concourse.bacc as bacc
import concourse.mybir as mybir
import concourse.tile as tile
from concourse import bass_utils

B, S, D = 8, 2048, 256
P = 128
NT = S // P  # 16 chunks of 128 along t (and s for output rows)
SB = 512  # moving-dim block for the big matmuls
NSB = S // SB  # 4
DC = D // P  # 2 partition chunks of the feature dim
NH = NT // 2  # n-chunks per half
SHIFT = 64.0  # global softmax shift

_CACHE = {}


def _build(reps: int = 1):
    f32, bf16, f16 = mybir.dt.float32, mybir.dt.bfloat16, mybir.dt.float16

    nc = bacc.Bacc("TRN2", target_bir_lowering=False, debug=False)
    enc_d = nc.dram_tensor("enc", [S, D], f32, kind="ExternalInput").ap()
    dec_d = nc.dram_tensor("dec", [S, D], f32, kind="ExternalInput").ap()
    w_d = nc.dram_tensor("w", [2 * D, D], f32, kind="ExternalInput").ap()
    out_d = nc.dram_tensor("out", [S, D], f32, kind="ExternalOutput").ap()

    with tile.TileContext(nc) as tc:
        with tc.tile_pool(name="big", bufs=1) as big:
            encT = big.tile([P, DC, S], f16, tag="encT")  # enc^T  (d-part, s-free)
            decT = big.tile([P, DC, S], f16, tag="decT")  # dec^T  (d-part, t-free)
            encN = big.tile([P, NT, D], bf16, tag="encN")  # enc natural, per t-chunk
            E = big.tile([P, NT, S], bf16, tag="E")  # exp(scoresT - SHIFT)
            U = big.tile([P, DC, S], bf16, tag="U")  # unnormalized context^T
            Wt1 = big.tile([P, DC, D], bf16, tag="Wt1")  # W_tanh rows 0..255 (ctx)
            Wt2 = big.tile([P, DC, D], f16, tag="Wt2")  # W_tanh rows 256..511 (dec)
            ones = big.tile([P, 1], f32, tag="ones")
            rden = big.tile([P, NT], f32, tag="rden")  # 1/denom, [s-part, s-chunk]
            nshift = big.tile([P, 1], f32, tag="nshift")
            outS = big.tile([P, NT, D], f32, tag="outS")  # staged output rows

            nc.any.memset(ones[:], 1.0)
            nc.any.memset(nshift[:], -SHIFT)

            for _rep in range(reps):
                _body(nc, tc, big, locals(), warmup=(_rep == 0))

    nc.compile()
    return nc


def _body(nc, tc, big, env, warmup=True):
    f32, bf16, f16 = mybir.dt.float32, mybir.dt.bfloat16, mybir.dt.float16
    AF = mybir.ActivationFunctionType
    ALU = mybir.AluOpType
    enc_d, dec_d, w_d, out_d = env["enc_d"], env["dec_d"], env["w_d"], env["out_d"]
    encT, decT, encN, E = env["encT"], env["decT"], env["encN"], env["E"]
    U, Wt1, Wt2 = env["U"], env["Wt1"], env["Wt2"]
    ones, rden, nshift, outS = env["ones"], env["rden"], env["nshift"], env["outS"]

    out_r = out_d.rearrange("(n p) d -> p n d", p=P)

    # ---- W: one batched DMA; rows 0..255 -> bf16 (ctx), 256..511 -> f16
    with tc.tile_pool(name="wstage", bufs=1) as wstage:
        wst = wstage.tile([P, 4, D], f32, tag="wst")
        nc.sync.dma_start(wst[:], w_d.rearrange("(r p) d -> p r d", p=P))
        for r in range(2):
            nc.vector.tensor_copy(Wt1[:, r, :], wst[:, r, :])
            nc.vector.tensor_copy(Wt2[:, r, :], wst[:, 2 + r, :])

    # ---- transposed operands: load f32 halves, cast to f16, bounce via
    # DRAM scratch, DMA-transpose per (half, d-chunk). enc natural is also
    # cast to bf16 for the context matmul.
    with (
        tc.tile_pool(name="scr", bufs=1, space="DRAM") as scr,
        tc.tile_pool(name="stS", bufs=3) as stS,
        tc.tile_pool(name="stH", bufs=3) as stH,
    ):
        scrE = scr.tile([S, D], f16, tag="scrE")
        scrD = scr.tile([S, D], f16, tag="scrD")
        for h in range(2):
            n0, r0 = h * NH, h * NH * P
            for src_d, scrX, dstT, natural in (
                (enc_d, scrE, encT, True),
                (dec_d, scrD, decT, False),
            ):
                xS = stS.tile([P, NH, D], f32, tag="xS")
                nc.sync.dma_start(
                    xS[:], src_d.rearrange("(n p) d -> p n d", p=P)[:, n0 : n0 + NH, :]
                )
                xH = stH.tile([P, NH, D], f16, tag="xH")
                nc.vector.tensor_copy(xH[:], xS[:])
                if natural:
                    nc.vector.tensor_copy(encN[:, n0 : n0 + NH, :], xS[:])
                nc.sync.dma_start(
                    scrX.rearrange("(n p) d -> p n d", p=P)[:, n0 : n0 + NH, :], xH[:]
                )
                for dc in range(DC):
                    nc.sync.dma_start(
                        out=dstT[:, dc, r0 : r0 + NH * P],
                        in_=scrX[r0 : r0 + NH * P, dc * P : (dc + 1) * P],
                        transpose=True,
                    )

    # ---- PE warmup: ~4us of dummy matmuls on W tiles during the input
    # DMA so the HAM clock-gate opens before the first real matmul.
    if warmup:
        with tc.tile_pool(name="ps_w", bufs=1, space="PSUM") as ps_w:
            pw = ps_w.tile([P, D], f32, tag="pw")
            for _ in range(20):
                nc.tensor.matmul(
                    pw[:], Wt1[:, 0, 0:P], Wt1[:, 0, :], start=True, stop=True
                )

    # ---- phase 1: scores + exp, t-outer. For each t-chunk: one LDW per
    # d-chunk covers 2 matmuls (two s-blocks into one 2-bank PSUM tile);
    # exp then covers 1024 columns per instruction.
    with (
        tc.tile_pool(name="ps_d", bufs=1, space="PSUM") as ps_d,
        tc.tile_pool(name="trP", bufs=6) as trP,
        tc.tile_pool(name="trQ", bufs=4) as trQ,
        tc.tile_pool(name="trR", bufs=4) as trR,
    ):
        pd = ps_d.tile([P, NT], f32, tag="pd")
        with tc.tile_pool(name="ps_s", bufs=2, space="PSUM") as ps_s:
            for t in range(NT):
                for half in range(2):
                    ps2 = ps_s.tile([P, 2, SB], f32, tag="ps2")
                    for dc in range(DC):
                        for sbh in range(2):
                            nc.tensor.matmul(
                                ps2[:, sbh, :],
                                decT[:, dc, t * P : (t + 1) * P],
                                encT[:, dc, (2 * half + sbh) * SB : (2 * half + sbh + 1) * SB],
                                start=(dc == 0),
                                stop=(dc == DC - 1),
                            )
                    nc.scalar.activation(
                        E[:, t, half * 2 * SB : (half + 1) * 2 * SB],
                        ps2[:, :, :],
                        AF.Exp,
                        bias=nshift[:],
                    )

        # denom tree over t-chunks, per s-half (FD=1024). Imbalanced so the
        # final combine after the last exp is short: A=sum(t0..7) closes at
        # t=7, B=sum(t8..13) at t=13, C=E14+E15 at t=15.
        HB = 2 * SB  # 1024
        esums = []
        for h in range(2):
            sl = slice(h * HB, (h + 1) * HB)
            l1 = []
            for i in range(7):
                p_i = trP.tile([P, HB], bf16, tag="p", name=f"p{h}_{i}")
                nc.vector.tensor_add(p_i[:], E[:, 2 * i, sl], E[:, 2 * i + 1, sl])
                l1.append(p_i)
            qa = trQ.tile([P, HB], f32, tag="q", name=f"qa{h}")
            nc.vector.tensor_add(qa[:], l1[0][:], l1[1][:])
            qb = trQ.tile([P, HB], f32, tag="q", name=f"qb{h}")
            nc.vector.tensor_add(qb[:], l1[2][:], l1[3][:])
            qc = trQ.tile([P, HB], f32, tag="q", name=f"qc{h}")
            nc.vector.tensor_add(qc[:], l1[4][:], l1[5][:])
            ra = trR.tile([P, HB], f32, tag="r", name=f"ra{h}")
            nc.vector.tensor_add(ra[:], qa[:], qb[:])  # sum t0..7, ready ~t=7
            rb = trR.tile([P, HB], f32, tag="r", name=f"rb{h}")
            nc.vector.tensor_add(rb[:], ra[:], qc[:])  # sum t0..11
            rc = trR.tile([P, HB], f32, tag="r", name=f"rc{h}")
            nc.vector.tensor_add(rc[:], rb[:], l1[6][:])  # sum t0..13
            # C = E14+E15 lands last; fold it in with one f32 add
            p_c = trP.tile([P, HB], bf16, tag="p", name=f"pc{h}")
            nc.vector.tensor_add(p_c[:], E[:, 14, sl], E[:, 15, sl])
            esum = trR.tile([P, HB], f32, tag="r", name=f"esum{h}")
            nc.vector.tensor_add(esum[:], rc[:], p_c[:])
            esums.append(esum)

        # fold across partitions (16 tiny f32 matmuls) + reciprocal
        for c in range(NT):
            h, cc = c // 8, c % 8
            nc.tensor.matmul(
                pd[:, c : c + 1],
                esums[h][:, cc * P : (cc + 1) * P],
                ones[:],
                start=True,
                stop=True,
            )
        nc.vector.reciprocal(rden[:], pd[:])

        # ---- phase 2: context U = encN^T @ E. One LDW per (pair, dc, t)
        # covers 2 matmuls (two s-blocks into one 2-bank PSUM tile).
        with tc.tile_pool(name="ps_u", bufs=2, space="PSUM") as ps_u:
            for pair in range(2):
                for dc in range(DC):
                    pu2 = ps_u.tile([P, 2, SB], f32, tag="pu2")
                    for t in range(NT):
                        for sbh in range(2):
                            nc.tensor.matmul(
                                pu2[:, sbh, :],
                                encN[:, t, dc * P : (dc + 1) * P],
                                E[:, t, (2 * pair + sbh) * SB : (2 * pair + sbh + 1) * SB],
                                start=(t == 0),
                                stop=(t == NT - 1),
                            )
                    nc.vector.tensor_copy(
                        U[:, dc, pair * 2 * SB : (pair + 1) * 2 * SB], pu2[:, :, :]
                    )

        # ---- phase 3: out = tanh(U^T@W1 * rden + dec@W2); tanh batched
        # over groups of 4 s-chunks, per-group output DMA.
        with (
            tc.tile_pool(name="ps_y", bufs=4, space="PSUM") as ps_y,
            tc.tile_pool(name="fout", bufs=2) as fout,
        ):
            for g in range(4):
                ft = fout.tile([P, 4, D], f32, tag="ft")
                for c4 in range(4):
                    c = g * 4 + c4
                    py = ps_y.tile([P, 2, D], f32, tag="py")
                    for dc in range(DC):
                        nc.tensor.matmul(
                            py[:, 0, :],
                            U[:, dc, c * P : (c + 1) * P],
                            Wt1[:, dc, :],
                            start=(dc == 0),
                            stop=(dc == DC - 1),
                        )
                    for dc in range(DC):
                        nc.tensor.matmul(
                            py[:, 1, :],
                            decT[:, dc, c * P : (c + 1) * P],
                            Wt2[:, dc, :],
                            start=(dc == 0),
                            stop=(dc == DC - 1),
                        )
                    t1 = fout.tile([P, D], f32, tag="t1", name="t1")
                    nc.vector.tensor_scalar_mul(t1[:], py[:, 0, :], rden[:, c : c + 1])
                    nc.vector.tensor_add(ft[:, c4, :], t1[:], py[:, 1, :])
                nc.scalar.activation(outS[:, g * 4 : (g + 1) * 4, :], ft[:, :, :], AF.Tanh)
                nc.sync.dma_start(
                    out_r[:, g * 4 : g * 4 + 4, :], outS[:, g * 4 : g * 4 + 4, :]
                )


def get_nc():
    if "nc" not in _CACHE:
        _CACHE["nc"] = _build()
    return _CACHE["nc"]


def _get_fn():
    """Build the sharded PJRT executable once and cache it; subsequent
    kernel() calls pay only input transfer + dispatch."""
    if "fn" in _CACHE:
        return _CACHE["fn"]
    import jax
    from jax.sharding import Mesh, NamedSharding, PartitionSpec
    from jax.experimental.shard_map import shard_map
    from concourse.bass2jax import (
        _bass_exec_p,
        install_neuronx_cc_hook,
        partition_id_tensor,
    )

    install_neuronx_cc_hook()
    nc = get_nc()
    out_avals = []
    for alloc in nc.m.functions[0].allocations:
        if (
            isinstance(alloc, mybir.MemoryLocationSet)
            and alloc.kind == "ExternalOutput"
        ):
            out_avals.append(
                jax.core.ShapedArray(
                    tuple(alloc.tensor_shape), mybir.dt.np(alloc.dtype)
                )
            )
    has_pid = nc.partition_id_tensor is not None
    names = ["enc", "dec", "w", "out"] + (["partition_id"] if has_pid else [])
    mesh = Mesh(np.asarray(jax.devices()[:B]), ("core",))
    spec = PartitionSpec("core")

    def _b(e, d, ww, z):
        ops = [e, d, ww, z] + ([partition_id_tensor()] if has_pid else [])
        return _bass_exec_p.bind(
            *ops,
            out_avals=tuple(out_avals),
            in_names=tuple(names),
            out_names=("out",),
            lowering_input_output_aliases=(),
            sim_require_finite=True,
            sim_require_nnan=True,
            nc=nc,
        )[0]

    jitted = jax.jit(
        shard_map(
            _b, mesh=mesh, in_specs=(spec,) * 4, out_specs=spec, check_rep=False
        ),
        donate_argnums=(3,),
        keep_unused=True,
    )
    sh = NamedSharding(mesh, spec)
    _CACHE["fn"] = (jitted, sh)
    return _CACHE["fn"]


def kernel(enc_outputs_top, dec_outputs_top, W_tanh):
    import jax

    enc = np.ascontiguousarray(enc_outputs_top, dtype=np.float32)
    dec = np.ascontiguousarray(dec_outputs_top, dtype=np.float32)
    w = np.ascontiguousarray(W_tanh, dtype=np.float32)
    try:
        fn, sh = _get_fn()
        eg = jax.device_put(enc.reshape(B * S, D), sh)
        dg = jax.device_put(dec.reshape(B * S, D), sh)
        wg = jax.device_put(np.concatenate([w] * B, axis=0), sh)
        zg = jax.device_put(np.zeros((B * S, D), np.float32), sh)
        out = np.asarray(jax.block_until_ready(fn(eg, dg, wg, zg)))
        return out.reshape(B, S, D)
    except Exception:
        # fallback: reference multi-core path (rebuilds the jit per call)
        nc = get_nc()
        in_maps = [{"enc": enc[b], "dec": dec[b], "w": w} for b in range(B)]
        res = bass_utils.run_bass_kernel_spmd(nc, in_maps, core_ids=list(range(B)))
        return np.stack([r["out"] for r in res.results], axis=0)


# revision 9
# speedup vs baseline: 2.8605x; 1.0940x over previous
"""Luong attention kernel for Trainium2 (Bass/Tile), data-parallel over batch.

Math (per batch b):
    scores[s,t] = enc[s,:] . dec[t,:]
    weights     = softmax(scores, axis=t)
    context[s]  = sum_t weights[s,t] * enc[t,:]
    out         = tanh(concat([context, dec]) @ W_tanh)

Implementation notes (v2):
  - B=8 batches -> 8 NeuronCores, one batch per core, no collectives.
  - scoresT[t,s] is computed (t on partitions) so the context contraction
    over t maps onto the PE (lhsT = enc natural, rhs = exp(scoresT)).
  - softmax uses a *global* shift (softmax is shift-invariant): E = exp(s-64).
    Scores ~ N(0, 256): row max is ~[45..95], so exp(s-64) stays inside
    bf16 range on both ends; E is kept unnormalized and 1/denom is applied
    in the output matmul epilogue, where denom is a per-partition scalar.
  - denom: E chunks are pairwise-summed with a DVE+GPSIMD tree (keeps the
    vector engine under the PE roofline), then folded across partitions
    with 16 tiny f32 PE matmuls against a ones vector -> [s-part, 1].
  - phase 3 fuses (y1 * rden + y2) into one scalar_tensor_tensor, then
    tanh on ACT; per-block output DMA overlaps the next block.
  - startup: enc/dec are loaded in halves, cast f32->f16 on DVE, bounced
    through DRAM scratch and DMA-transposed per half so the first scores
    matmuls start while the tail of the input is still in flight.
"""

import sys

if "/opt/trn_rl_repo" not in sys.path:
    sys.path.insert(0, "/opt/trn_rl_repo")

import numpy as np

import concourse.bacc as bacc
import concourse.mybir as mybir
import concourse.tile as tile
from concourse import bass_utils

B, S, D = 8, 2048, 256
P = 128
NT = S // P  # 16 chunks of 128 along t (and s for output rows)
SB = 512  # moving-dim block for the big matmuls
NSB = S // SB  # 4
DC = D // P  # 2 partition chunks of the feature dim
NH = NT // 2  # n-chunks per half
SHIFT = 64.0  # global softmax shift

_CACHE = {}


def _build(reps: int = 1):
    f32, bf16, f16 = mybir.dt.float32, mybir.dt.bfloat16, mybir.dt.float16

    nc = bacc.Bacc("TRN2", target_bir_lowering=False, debug=False)
    enc_d = nc.dram_tensor("enc", [S, D], f32, kind="ExternalInput").ap()
    dec_d = nc.dram_tensor("dec", [S, D], f32, kind="ExternalInput").ap()
    w_d = nc.dram_tensor("w", [2 * D, D], f32, kind="ExternalInput").ap()
    out_d = nc.dram_tensor("out", [S, D], f32, kind="ExternalOutput").ap()

    with tile.TileContext(nc) as tc:
        with tc.tile_pool(name="big", bufs=1) as big:
            encT = big.tile([P, DC, S], f16, tag="encT")  # enc^T  (d-part, s-free)
            decT = big.tile([P, DC, S], f16, tag="decT")  # dec^T  (d-part, t-free)
            encN = big.tile([P, NT, D], bf16, tag="encN")  # enc natural, per t-chunk
            E = big.tile([P, NT, S], bf16, tag="E")  # exp(scoresT - SHIFT)
            U = big.tile([P, DC, S], bf16, tag="U")  # unnormalized context^T
            Wt1 = big.tile([P, DC, D], bf16, tag="Wt1")  # W_tanh rows 0..255 (ctx)
            Wt2 = big.tile([P, DC, D], f16, tag="Wt2")  # W_tanh rows 256..511 (dec)
            ones = big.tile([P, 1], f32, tag="ones")
            rden = big.tile([P, NT], f32, tag="rden")  # 1/denom, [s-part, s-chunk]
            nshift = big.tile([P, 1], f32, tag="nshift")
            outS = big.tile([P, NT, D], f32, tag="outS")  # staged output rows

            nc.any.memset(ones[:], 1.0)
            nc.any.memset(nshift[:], -SHIFT)

            for _rep in range(reps):
                _body(nc, tc, big, locals(), warmup=(_rep == 0))

    nc.compile()
    return nc


def _body(nc, tc, big, env, warmup=True):
    f32, bf16, f16 = mybir.dt.float32, mybir.dt.bfloat16, mybir.dt.float16
    AF = mybir.ActivationFunctionType
    ALU = mybir.AluOpType
    enc_d, dec_d, w_d, out_d = env["enc_d"], env["dec_d"], env["w_d"], env["out_d"]
    encT, decT, encN, E = env["encT"], env["decT"], env["encN"], env["E"]
    U, Wt1, Wt2 = env["U"], env["Wt1"], env["Wt2"]
    ones, rden, nshift, outS = env["ones"], env["rden"], env["nshift"], env["outS"]

    out_r = out_d.rearrange("(n p) d -> p n d", p=P)

    # ---- W: one batched DMA; rows 0..255 -> bf16 (ctx), 256..511 -> f16
    with tc.tile_pool(name="wstage", bufs=1) as wstage:
        wst = wstage.tile([P, 4, D], f32, tag="wst")
        nc.sync.dma_start(wst[:], w_d.rearrange("(r p) d -> p r d", p=P))
        for r in range(2):
            nc.vector.tensor_copy(Wt1[:, r, :], wst[:, r, :])
            nc.vector.tensor_copy(Wt2[:, r, :], wst[:, 2 + r, :])

    # ---- transposed operands: load f32 halves, cast to f16, bounce via
    # DRAM scratch, DMA-transpose per (half, d-chunk). enc natural is also
    # cast to bf16 for the context matmul.
    with (
        tc.tile_pool(name="scr", bufs=1, space="DRAM") as scr,
        tc.tile_pool(name="stS", bufs=3) as stS,
        tc.tile_pool(name="stH", bufs=3) as stH,
    ):
        scrE = scr.tile([S, D], f16, tag="scrE")
        scrD = scr.tile([S, D], f16, tag="scrD")
        for h in range(2):
            n0, r0 = h * NH, h * NH * P
            for src_d, scrX, dstT, natural in (
                (enc_d, scrE, encT, True),
                (dec_d, scrD, decT, False),
            ):
                xS = stS.tile([P, NH, D], f32, tag="xS")
                nc.sync.dma_start(
                    xS[:], src_d.rearrange("(n p) d -> p n d", p=P)[:, n0 : n0 + NH, :]
                )
                xH = stH.tile([P, NH, D], f16, tag="xH")
                nc.vector.tensor_copy(xH[:], xS[:])
                if natural:
                    nc.vector.tensor_copy(encN[:, n0 : n0 + NH, :], xS[:])
                nc.sync.dma_start(
                    scrX.rearrange("(n p) d -> p n d", p=P)[:, n0 : n0 + NH, :], xH[:]
                )
                for dc in range(DC):
                    nc.sync.dma_start(
                        out=dstT[:, dc, r0 : r0 + NH * P],
                        in_=scrX[r0 : r0 + NH * P, dc * P : (dc + 1) * P],
                        transpose=True,
                    )

    # ---- PE warmup: ~4us of dummy matmuls on W tiles during the input
    # DMA so the HAM clock-gate opens before the first real matmul.
    if warmup:
        with tc.tile_pool(name="ps_w", bufs=1, space="PSUM") as ps_w:
            pw = ps_w.tile([P, D], f32, tag="pw")
            for _ in range(20):
                nc.tensor.matmul(
                    pw[:], Wt1[:, 0, 0:P], Wt1[:, 0, :], start=True, stop=True
                )

    # ---- main loop: per s-block of 512: scores -> exp -> denom tree;
    # context; output epilogue. Block sb's epilogue overlaps sb+1's scores.
    with (
        tc.tile_pool(name="ps_s", bufs=3, space="PSUM") as ps_s,
        tc.tile_pool(name="ps_u", bufs=2, space="PSUM") as ps_u,
        tc.tile_pool(name="ps_y", bufs=2, space="PSUM") as ps_y,
        tc.tile_pool(name="ps_d", bufs=1, space="PSUM") as ps_d,
        tc.tile_pool(name="trP", bufs=6) as trP,
        tc.tile_pool(name="trQ", bufs=4) as trQ,
        tc.tile_pool(name="trR", bufs=3) as trR,
        tc.tile_pool(name="fout", bufs=2) as fout,
    ):
        pd = ps_d.tile([P, NT], f32, tag="pd")
        for sb in range(NSB):
            s_lo, s_hi = sb * SB, (sb + 1) * SB
            # scores + exp, one [128,512] PSUM bank per t-chunk
            for t in range(NT):
                ps = ps_s.tile([P, SB], f32, tag="ps")
                for dc in range(DC):
                    nc.tensor.matmul(
                        ps[:],
                        decT[:, dc, t * P : (t + 1) * P],
                        encT[:, dc, s_lo:s_hi],
                        start=(dc == 0),
                        stop=(dc == DC - 1),
                    )
                nc.scalar.activation(
                    E[:, t, s_lo:s_hi], ps[:], AF.Exp, bias=nshift[:]
                )
            # denom tree, all DVE: bf16 pairwise levels then f32 combine
            l1 = []
            for i in range(8):
                p_i = trP.tile([P, SB], bf16, tag="p", name=f"p{i}")
                nc.vector.tensor_add(
                    p_i[:], E[:, 2 * i, s_lo:s_hi], E[:, 2 * i + 1, s_lo:s_hi]
                )
                l1.append(p_i)
            l2 = []
            for j in range(4):
                q_j = trQ.tile([P, SB], bf16, tag="q", name=f"q{j}")
                nc.vector.tensor_add(q_j[:], l1[2 * j][:], l1[2 * j + 1][:])
                l2.append(q_j)
            r0 = trR.tile([P, SB], f32, tag="r", name="r0")
            nc.vector.tensor_add(r0[:], l2[0][:], l2[1][:])
            r1 = trR.tile([P, SB], f32, tag="r", name="r1")
            nc.vector.tensor_add(r1[:], l2[2][:], l2[3][:])
            esum = trR.tile([P, SB], f32, tag="r", name="esum")
            nc.vector.tensor_add(esum[:], r0[:], r1[:])
            # fold across partitions (4 tiny f32 matmuls) + reciprocal
            for c in range(4):
                nc.tensor.matmul(
                    pd[:, sb * 4 + c : sb * 4 + c + 1],
                    esum[:, c * P : (c + 1) * P],
                    ones[:],
                    start=True,
                    stop=True,
                )
            nc.vector.reciprocal(
                rden[:, sb * 4 : sb * 4 + 4], pd[:, sb * 4 : sb * 4 + 4]
            )
            # context
            for dc in range(DC):
                pu = ps_u.tile([P, SB], f32, tag="pu")
                for t in range(NT):
                    nc.tensor.matmul(
                        pu[:],
                        encN[:, t, dc * P : (dc + 1) * P],
                        E[:, t, s_lo:s_hi],
                        start=(t == 0),
                        stop=(t == NT - 1),
                    )
                nc.vector.tensor_copy(U[:, dc, s_lo:s_hi], pu[:])
            # output epilogue; tanh batched over the block's 4 s-chunks
            ft = fout.tile([P, 4, D], f32, tag="ft")
            for c4 in range(4):
                c = sb * 4 + c4
                py = ps_y.tile([P, 2, D], f32, tag="py")
                for dc in range(DC):
                    nc.tensor.matmul(
                        py[:, 0, :],
                        U[:, dc, c * P : (c + 1) * P],
                        Wt1[:, dc, :],
                        start=(dc == 0),
                        stop=(dc == DC - 1),
                    )
                for dc in range(DC):
                    nc.tensor.matmul(
                        py[:, 1, :],
                        decT[:, dc, c * P : (c + 1) * P],
                        Wt2[:, dc, :],
                        start=(dc == 0),
                        stop=(dc == DC - 1),
                    )
                t1 = fout.tile([P, D], f32, tag="t1", name="t1")
                nc.vector.tensor_scalar_mul(t1[:], py[:, 0, :], rden[:, c : c + 1])
                nc.vector.tensor_add(ft[:, c4, :], t1[:], py[:, 1, :])
            nc.scalar.activation(outS[:, sb * 4 : sb * 4 + 4, :], ft[:, :, :], AF.Tanh)
            nc.sync.dma_start(
                out_r[:, sb * 4 : sb * 4 + 4, :], outS[:, sb * 4 : sb * 4 + 4, :]
            )


def get_nc():
    if "nc" not in _CACHE:
        _CACHE["nc"] = _build()
    return _CACHE["nc"]


def _get_fn():
    """Build the sharded PJRT executable once and cache it; subsequent
    kernel() calls pay only input transfer + dispatch."""
    if "fn" in _CACHE:
        return _CACHE["fn"]
    import jax
    from jax.sharding import Mesh, NamedSharding, PartitionSpec
    from jax.experimental.shard_map import shard_map
    from concourse.bass2jax import (
        _bass_exec_p,
        install_neuronx_cc_hook,
        partition_id_tensor,
    )

    install_neuronx_cc_hook()
    nc = get_nc()
    out_avals = []
    for alloc in nc.m.functions[0].allocations:
        if (
            isinstance(alloc, mybir.MemoryLocationSet)
            and alloc.kind == "ExternalOutput"
        ):
            out_avals.append(
                jax.core.ShapedArray(
                    tuple(alloc.tensor_shape), mybir.dt.np(alloc.dtype)
                )
            )
    has_pid = nc.partition_id_tensor is not None
    names = ["enc", "dec", "w", "out"] + (["partition_id"] if has_pid else [])
    mesh = Mesh(np.asarray(jax.devices()[:B]), ("core",))
    spec = PartitionSpec("core")

    def _b(e, d, ww, z):
        ops = [e, d, ww, z] + ([partition_id_tensor()] if has_pid else [])
        return _bass_exec_p.bind(
            *ops,
            out_avals=tuple(out_avals),
            in_names=tuple(names),
            out_names=("out",),
            lowering_input_output_aliases=(),
            sim_require_finite=True,
            sim_require_nnan=True,
            nc=nc,
        )[0]

    jitted = jax.jit(
        shard_map(
            _b, mesh=mesh, in_specs=(spec,) * 4, out_specs=spec, check_rep=False
        ),
        donate_argnums=(3,),
        keep_unused=True,
    )
    sh = NamedSharding(mesh, spec)
    _CACHE["fn"] = (jitted, sh)
    return _CACHE["fn"]


def kernel(enc_outputs_top, dec_outputs_top, W_tanh):
    import jax

    enc = np.ascontiguousarray(enc_outputs_top, dtype=np.float32)
    dec = np.ascontiguousarray(dec_outputs_top, dtype=np.float32)
    w = np.ascontiguousarray(W_tanh, dtype=np.float32)
    try:
        fn, sh = _get_fn()
        eg = jax.device_put(enc.reshape(B * S, D), sh)
        dg = jax.device_put(dec.reshape(B * S, D), sh)
        wg = jax.device_put(np.concatenate([w] * B, axis=0), sh)
        zg = jax.device_put(np.zeros((B * S, D), np.float32), sh)
        out = np.asarray(jax.block_until_ready(fn(eg, dg, wg, zg)))
        return out.reshape(B, S, D)
    except Exception:
        # fallback: reference multi-core path (rebuilds the jit per call)
        nc = get_nc()
        in_maps = [{"enc": enc[b], "dec": dec[b], "w": w} for b in range(B)]
        res = bass_utils.run_bass_kernel_spmd(nc, in_maps, core_ids=list(range(B)))
        return np.stack([r["out"] for r in res.results], axis=0)
